# revision 33
# baseline (speedup 1.0000x reference)
"""AGCRN cell kernel for 8 Trainium2 NeuronCores.

Strategy: data-parallel over batch (B=32 -> 4 per core, no collectives).
Each core redundantly builds S = exp(relu(E E^T)) (symmetric, so it serves
directly as the chain matmul's stationary operand without any transpose)
plus row sums d; the adaptive-adjacency normalization 1/d is folded into
PSUM evacuations as a per-partition scale. Logits are computed at full f32
precision via a split-bf16 matmul ([Eh;El;Eh]^T [Eh;Eh;El], K=30). The
Chebyshev chain runs node-major with 264-wide moving operands (4 batches x
66 ch); the x_g @ W contraction transposes 68-column chunks on the PE
(zero-padded W rows absorb pad garbage, a ones-column provides the bias
for free), and each node-tile's tail work is emitted interleaved with the
second chain application so it rides inside the PE matmul stream.
Measured: 229.4 us on hardware, rel err 3.5e-3 (gate 2e-2).
"""

import os
import sys

import numpy as np
import ml_dtypes

for _p in ("/opt/trn_rl_repo", "/root/.axon_site/_ro/trn_rl_repo"):
    if os.path.isdir(_p) and _p not in sys.path:
        sys.path.append(_p)

import concourse.bass as bass
import concourse.tile as tile
from concourse import bacc, mybir
from concourse.bass_utils import run_bass_kernel_spmd
from concourse.masks import make_identity

F32 = mybir.dt.float32
BF16 = mybir.dt.bfloat16
AF = mybir.ActivationFunctionType
ALU = mybir.AluOpType

P = 128          # partitions
N = 2048         # nodes
NT = N // P      # node tiles = 16
NB = 4           # batches per core
CH = 66          # dim_in + hidden
CPB = 96         # padded channel slot per batch (66 real + 1 ones + pad)
HID = 64
OC_G = 128       # gate output channels (2*hidden)
NCORES = 8
RT_GROUP = 16    # row-tiles per transpose/matmul group
TW = 68          # transposed-chunk rows (66 channels + ones column + pad)


def _dv(ap, nb=NB, w=CPB):
    """View a [P, nb*w] slice as [P, nb, w]."""
    return ap.rearrange("p (b c) -> p b c", b=nb)


def build_nc():
    nc = bacc.Bacc(
        "TRN2",
        target_bir_lowering=False,
        debug=False,
        enable_asserts=False,
        num_devices=NCORES,
    )
    x_d = nc.dram_tensor("x", [NB, N, 2], F32, kind="ExternalInput").ap()
    st_d = nc.dram_tensor("state", [NB, N, HID], F32, kind="ExternalInput").ap()
    # [E_hi; E_lo; E_hi] — with rhs [E_hi; E_hi; E_lo] gives
    # Eh·Eh + El·Eh + Eh·El ≈ E·E in full f32 precision (bf16 products are
    # exact in f32 accumulation; the dropped El·El term is ~1e-5 relative).
    et_d = nc.dram_tensor("et", [2, 30, N], BF16, kind="ExternalInput").ap()
    id_d = nc.dram_tensor("ident", [P, P], BF16, kind="ExternalInput").ap()
    wg_d = nc.dram_tensor("wg", [3, TW, OC_G], BF16, kind="ExternalInput").ap()
    wu_d = nc.dram_tensor("wu", [3, TW, HID], BF16, kind="ExternalInput").ap()
    out_d = nc.dram_tensor("out", [NB, N, HID], F32, kind="ExternalOutput").ap()

    with tile.TileContext(nc) as tc:
        _build(tc, x_d, st_d, et_d, id_d, wg_d, wu_d, out_d)
    nc.compile()
    return nc


def _build(tc, x_d, st_d, et_d, id_d, wg_d, wu_d, out_d):
    nc = tc.nc
    from contextlib import ExitStack

    with ExitStack() as ctx:
        const = ctx.enter_context(tc.tile_pool(name="const", bufs=1))
        persist = ctx.enter_context(tc.tile_pool(name="persist", bufs=1))

        ident = const.tile([P, P], BF16)
        nc.sync.dma_start(ident[:], id_d[:])

        # ~3.5us of dummy matmuls: pushes the PE HAM clock-gate to 8/8
        # before the real work arrives
        with tc.tile_pool(name="warm", bufs=1, space="PSUM") as warm:
            wp = warm.tile([P, P], F32)
            for _ in range(32):
                nc.tensor.matmul(wp[:], lhsT=ident[:], rhs=ident[:], start=True, stop=True)

        etp = const.tile([30, 2, N], BF16)
        nc.scalar.dma_start(etp[:, 0, :], et_d[0])
        nc.scalar.dma_start(etp[:, 1, :], et_d[1])
        wg_sb = const.tile([TW, 3, OC_G], BF16)
        wu_sb = const.tile([TW, 3, HID], BF16)
        for k in range(3):
            nc.scalar.dma_start(wg_sb[:, k, :], wg_d[k])
            nc.scalar.dma_start(wu_sb[:, k, :], wu_d[k])

        S_sb = persist.tile([P, NT, N], BF16)       # S row-tiles
        x0_sb = persist.tile([P, NT, NB * CPB], BF16)
        u1_sb = persist.tile([P, NT, NB * CPB], BF16)
        u2_sb = persist.tile([P, NT, NB * CPB], BF16)
        stt_sb = persist.tile([P, NT, NB, HID], BF16)   # state copy for epilogue
        zr_sb = persist.tile([P, NT, NB, OC_G], BF16)   # sigmoid(gate)
        dtot = persist.tile([P, NT], F32)
        rinv = persist.tile([P, NT], F32)
        rinv2 = persist.tile([P, NT], F32)

        # ---- input load: one big strided DMA per batch (conversions are
        # emitted after the S-build so they don't block DVE's relu work) ----
        inp_pool = ctx.enter_context(tc.tile_pool(name="inp", bufs=1))
        stfs, xfs = [], []
        for b in range(NB):
            stf = inp_pool.tile([P, NT, HID], F32, tag=f"stf{b}")
            xf = inp_pool.tile([P, NT, 2], F32, tag=f"xf{b}")
            nc.sync.dma_start(stf[:], st_d[b].rearrange("(t p) h -> p t h", p=P))
            nc.sync.dma_start(xf[:], x_d[b].rearrange("(t p) h -> p t h", p=P))
            stfs.append(stf)
            xfs.append(xf)

        # ---- S = exp(relu(E E^T)) with row sums ----
        with (
            tc.tile_pool(name="lpsum", bufs=3, space="PSUM") as lpsum,
            tc.tile_pool(name="lrelu", bufs=3) as lrelu,
        ):
            for mt in range(NT):
                lr = lrelu.tile([P, N], F32)
                for q in range(4):
                    lp = lpsum.tile([P, 512], F32)
                    nc.tensor.matmul(
                        lp[:],
                        lhsT=etp[:, 0, mt * P : (mt + 1) * P],
                        rhs=etp[:, 1, q * 512 : (q + 1) * 512],
                        start=True,
                        stop=True,
                    )
                    nc.vector.tensor_scalar_max(
                        lr[:, q * 512 : (q + 1) * 512], lp[:], 0.0
                    )
                # exp + row-sum in one big ACT op
                nc.scalar.activation(
                    S_sb[:, mt, :], lr[:], AF.Exp,
                    accum_out=dtot[:, mt : mt + 1],
                )
                nc.vector.reciprocal(rinv[:, mt : mt + 1], dtot[:, mt : mt + 1])
                nc.vector.tensor_scalar_mul(
                    rinv2[:, mt : mt + 1], rinv[:, mt : mt + 1], 2.0
                )

        nc.gpsimd.memset(x0_sb[:], 0.0)
        nc.gpsimd.memset(u1_sb[:], 0.0)
        nc.gpsimd.memset(u2_sb[:], 0.0)
        for b in range(NB):
            # ones column feeding the bias row of W chunk 2
            nc.gpsimd.memset(u2_sb[:, :, b * CPB + CH : b * CPB + CH + 1], 1.0)

        for b in range(NB):
            nc.vector.tensor_copy(
                x0_sb[:, :, b * CPB + 2 : b * CPB + 2 + HID], stfs[b][:]
            )
            nc.vector.tensor_copy(x0_sb[:, :, b * CPB : b * CPB + 2], xfs[b][:])
            nc.vector.tensor_copy(stt_sb[:, :, b, :], stfs[b][:])

        cpsum = ctx.enter_context(tc.tile_pool(name="cpsum", bufs=2, space="PSUM"))
        tpsum = ctx.enter_context(tc.tile_pool(name="tpsum", bufs=3, space="PSUM"))
        zpsum = ctx.enter_context(tc.tile_pool(name="zpsum", bufs=2, space="PSUM"))
        xgt_pool = ctx.enter_context(tc.tile_pool(name="xgt", bufs=3 * RT_GROUP))
        epi_pool = ctx.enter_context(tc.tile_pool(name="epi", bufs=6))

        def apply_S(src, dst, second, only_mt=None):
            """dst = (S @ src) / d   (or 2*(S @ src)/d - x0 when second)."""
            mts = range(NT) if only_mt is None else [only_mt]
            for mt in mts:
                cp = cpsum.tile([P, NB * CH], F32)
                for kt in range(NT):
                    nc.tensor.matmul(
                        cp[:],
                        lhsT=S_sb[:, kt, mt * P : (mt + 1) * P],
                        rhs=_dv(src[:, kt, :])[:, :, 0:CH],
                        start=(kt == 0),
                        stop=(kt == NT - 1),
                    )
                dstv = _dv(dst[:, mt, :])[:, :, 0:CH]
                if not second:
                    nc.scalar.activation(
                        dstv, cp[:], AF.Copy, scale=rinv[:, mt : mt + 1]
                    )
                else:
                    nc.vector.scalar_tensor_tensor(
                        out=dstv,
                        in0=cp[:],
                        scalar=rinv2[:, mt : mt + 1],
                        in1=_dv(x0_sb[:, mt, :])[:, :, 0:CH],
                        op0=ALU.mult,
                        op1=ALU.subtract,
                    )

        def tail_nt(nt, gate):
            """Transposes + W matmul + nonlinearity (+ epilogue) for one
            node tile (all 4 batches, batched PSUM + single wide ops)."""
            w_sb = wg_sb if gate else wu_sb
            oc = OC_G if gate else HID
            xgts = {}
            for b in range(NB):
                tp = tpsum.tile([TW, 3, P], BF16, tag="tp", name=f"tp{nt}{b}")
                for k, srcb in enumerate((x0_sb, u1_sb, u2_sb)):
                    nc.tensor.transpose(
                        tp[:, k, :],
                        srcb[:, nt, b * CPB : b * CPB + TW],
                        ident[:],
                    )
                xgt = xgt_pool.tile([TW, 3, P], BF16, tag="xgt", name=f"xg{nt}{b}")
                nc.vector.tensor_copy(xgt[:], tp[:])
                xgts[b] = xgt
            zp = zpsum.tile([P, NB, oc], F32, tag="zp", name=f"zp{nt}")
            for b in range(NB):
                for k in range(3):
                    nc.tensor.matmul(
                        zp[:, b, :],
                        lhsT=xgts[b][:, k, :],
                        rhs=w_sb[:, k, :],
                        start=(k == 0),
                        stop=(k == 2),
                    )
            if gate:
                nc.scalar.activation(zr_sb[:, nt], zp[:], AF.Sigmoid)
                # candidate: state-cols of x0 *= z  (in place, all b)
                x0c = _dv(x0_sb[:, nt, :])[:, :, 2 : 2 + HID]
                nc.vector.tensor_mul(x0c, x0c, zr_sb[:, nt, :, 0:HID])
            else:
                hc = epi_pool.tile([P, NB, HID], BF16, tag="hc", name=f"hc{nt}")
                nc.scalar.activation(hc[:], zp[:], AF.Tanh)
                r = zr_sb[:, nt, :, HID:OC_G]
                t1 = epi_pool.tile([P, NB, HID], BF16, tag="t1", name=f"t1{nt}")
                nc.vector.tensor_sub(t1[:], stt_sb[:, nt], hc[:])
                hf = epi_pool.tile([P, NB, HID], F32, tag="hf", name=f"hf{nt}")
                # h = hc + r*(state - hc)
                nc.vector.scalar_tensor_tensor(
                    out=hf[:], in0=t1[:], scalar=1.0, in1=r,
                    op0=ALU.mult, op1=ALU.mult,
                )
                nc.vector.tensor_add(hf[:], hf[:], hc[:])
                nc.sync.dma_start(
                    out_d[:, nt * P : (nt + 1) * P, :].rearrange(
                        "b p h -> p b h"
                    ),
                    hf[:],
                )

        # each gconv: first chain application, then the second application
        # interleaved per-mt with that node-tile's transpose/W-matmul tail
        # so the tail rides inside the chain's PE stream
        for gate in (True, False):
            apply_S(x0_sb, u1_sb, second=False)
            for mt in range(NT):
                apply_S(u1_sb, u2_sb, second=True, only_mt=mt)
                tail_nt(mt, gate)


_NC = None


def _get_nc():
    global _NC
    if _NC is None:
        _NC = build_nc()
    return _NC


def _prep_in_maps(x, state, node_embeddings, W_gate, b_gate, W_update, b_update):
    bf = ml_dtypes.bfloat16
    x = np.asarray(x, dtype=np.float32)
    state = np.asarray(state, dtype=np.float32)
    E = np.asarray(node_embeddings, dtype=np.float32)
    W_gate = np.asarray(W_gate, dtype=np.float32)
    b_gate = np.asarray(b_gate, dtype=np.float32)
    W_update = np.asarray(W_update, dtype=np.float32)
    b_update = np.asarray(b_update, dtype=np.float32)

    eh = E.T.astype(bf)                       # [10, N] bf16
    el = (E.T - eh.astype(np.float32)).astype(bf)
    stack_l = np.concatenate([eh, el, eh], axis=0)   # lhsT rows
    stack_r = np.concatenate([eh, eh, el], axis=0)   # rhs rows
    et = np.ascontiguousarray(np.stack([stack_l, stack_r]))  # [2, 30, N]
    wg = np.zeros((3, 68, OC_G), np.float32)
    wu = np.zeros((3, 68, HID), np.float32)
    for k in range(3):
        wg[k, :CH] = W_gate[CH * k : CH * (k + 1)]
        wu[k, :CH] = W_update[CH * k : CH * (k + 1)]
    wg[2, CH] = b_gate
    wu[2, CH] = b_update
    wg = wg.astype(bf)
    wu = wu.astype(bf)

    ident = np.eye(P, dtype=bf)
    in_maps = []
    for r in range(NCORES):
        in_maps.append(
            {
                "x": np.ascontiguousarray(x[NB * r : NB * (r + 1)]),
                "state": np.ascontiguousarray(state[NB * r : NB * (r + 1)]),
                "et": et,
                "ident": ident,
                "wg": wg,
                "wu": wu,
            }
        )
    return in_maps


def run(trace=False, **inputs):
    nc = _get_nc()
    in_maps = _prep_in_maps(**inputs)
    res = run_bass_kernel_spmd(
        nc, in_maps, core_ids=list(range(NCORES)), trace=trace
    )
    out = np.concatenate([res.results[r]["out"] for r in range(NCORES)], axis=0)
    return out, res


def kernel(**inputs) -> np.ndarray:
    out, _ = run(trace=False, **inputs)
    return out


# revision 35
# speedup vs baseline: 1.1571x; 1.1571x over previous
"""AGCRN cell kernel for 8 Trainium2 NeuronCores.

Strategy: data-parallel over batch (B=32 -> 4 per core, no collectives).
Each core redundantly builds S = exp(relu(E E^T)) (symmetric, so it serves
directly as the chain matmul's stationary operand without any transpose)
plus row sums d; the adaptive-adjacency normalization 1/d is folded into
PSUM evacuations as a per-partition scale. Logits are computed at full f32
precision via a split-bf16 matmul ([Eh;El;Eh]^T [Eh;Eh;El], K=30). The
Chebyshev chain runs node-major with 264-wide moving operands (4 batches x
66 ch); the x_g @ W contraction transposes 68-column chunks on the PE
(zero-padded W rows absorb pad garbage, a ones-column provides the bias
for free), and each node-tile's tail work is emitted interleaved with the
second chain application so it rides inside the PE matmul stream.
Measured: 229.4 us on hardware, rel err 3.5e-3 (gate 2e-2).
"""

import os
import sys

import numpy as np
import ml_dtypes

for _p in ("/opt/trn_rl_repo", "/root/.axon_site/_ro/trn_rl_repo"):
    if os.path.isdir(_p) and _p not in sys.path:
        sys.path.append(_p)

import concourse.bass as bass
import concourse.tile as tile
from concourse import bacc, mybir
from concourse.bass_utils import run_bass_kernel_spmd
from concourse.masks import make_identity

F32 = mybir.dt.float32
BF16 = mybir.dt.bfloat16
AF = mybir.ActivationFunctionType
ALU = mybir.AluOpType

P = 128          # partitions
N = 2048         # nodes
NT = N // P      # node tiles = 16
NB = 4           # batches per core
CH = 66          # dim_in + hidden
CPB = 96         # padded channel slot per batch (66 real + 1 ones + pad)
HID = 64
OC_G = 128       # gate output channels (2*hidden)
NCORES = 8
RT_GROUP = 16    # row-tiles per transpose/matmul group
TW = 68          # transposed-chunk rows (66 channels + ones column + pad)


def _dv(ap, nb=NB, w=CPB):
    """View a [P, nb*w] slice as [P, nb, w]."""
    return ap.rearrange("p (b c) -> p b c", b=nb)


def build_nc():
    nc = bacc.Bacc(
        "TRN2",
        target_bir_lowering=False,
        debug=False,
        enable_asserts=False,
        num_devices=NCORES,
    )
    x_d = nc.dram_tensor("x", [NB, N, 2], F32, kind="ExternalInput").ap()
    st_d = nc.dram_tensor("state", [NB, N, HID], F32, kind="ExternalInput").ap()
    # [E_hi; E_lo; E_hi] — with rhs [E_hi; E_hi; E_lo] gives
    # Eh·Eh + El·Eh + Eh·El ≈ E·E in full f32 precision (bf16 products are
    # exact in f32 accumulation; the dropped El·El term is ~1e-5 relative).
    et_d = nc.dram_tensor("et", [2, 30, N], BF16, kind="ExternalInput").ap()
    id_d = nc.dram_tensor("ident", [P, P], BF16, kind="ExternalInput").ap()
    wg_d = nc.dram_tensor("wg", [3, TW, OC_G], BF16, kind="ExternalInput").ap()
    wu_d = nc.dram_tensor("wu", [3, TW, HID], BF16, kind="ExternalInput").ap()
    out_d = nc.dram_tensor("out", [NB, N, HID], F32, kind="ExternalOutput").ap()

    with tile.TileContext(nc) as tc:
        _build(tc, x_d, st_d, et_d, id_d, wg_d, wu_d, out_d)
    nc.compile()
    return nc


def _build(tc, x_d, st_d, et_d, id_d, wg_d, wu_d, out_d):
    nc = tc.nc
    from contextlib import ExitStack

    with ExitStack() as ctx:
        const = ctx.enter_context(tc.tile_pool(name="const", bufs=1))
        persist = ctx.enter_context(tc.tile_pool(name="persist", bufs=1))

        ident = const.tile([P, P], BF16)
        nc.sync.dma_start(ident[:], id_d[:])

        # ~3.5us of dummy matmuls: pushes the PE HAM clock-gate to 8/8
        # before the real work arrives
        with tc.tile_pool(name="warm", bufs=1, space="PSUM") as warm:
            wp = warm.tile([P, P], F32)
            for _ in range(32):
                nc.tensor.matmul(wp[:], lhsT=ident[:], rhs=ident[:], start=True, stop=True)

        etp = const.tile([30, 2, N], BF16)
        nc.scalar.dma_start(etp[:, 0, :], et_d[0])
        nc.scalar.dma_start(etp[:, 1, :], et_d[1])
        wg_sb = const.tile([TW, 3, OC_G], BF16)
        wu_sb = const.tile([TW, 3, HID], BF16)
        for k in range(3):
            nc.scalar.dma_start(wg_sb[:, k, :], wg_d[k])
            nc.scalar.dma_start(wu_sb[:, k, :], wu_d[k])

        S_sb = persist.tile([P, NT, N], BF16)       # S row-tiles
        x0_sb = persist.tile([P, NT, NB * CPB], BF16)
        u1_sb = persist.tile([P, NT, NB * CPB], BF16)
        u2_sb = persist.tile([P, NT, NB * CPB], BF16)
        stt_sb = persist.tile([P, NT, NB, HID], BF16)   # state copy for epilogue
        zr_sb = persist.tile([P, NT, NB, OC_G], BF16)   # sigmoid(gate)
        dtot = persist.tile([P, NT], F32)
        rinv = persist.tile([P, NT], F32)
        rinv2 = persist.tile([P, NT], F32)

        # zero only the pad columns the transposes read (cols 66..96 of
        # each batch slot); real columns are fully written before any read
        for buf in (x0_sb, u1_sb, u2_sb):
            nc.gpsimd.memset(
                buf.rearrange("p t (b c) -> p t b c", b=NB)[:, :, :, CH:CPB], 0.0
            )
        for b in range(NB):
            # ones column feeding the bias row of W chunk 2
            nc.gpsimd.memset(u2_sb[:, :, b * CPB + CH : b * CPB + CH + 1], 1.0)

        # ---- input load: one big strided DMA per batch (conversions are
        # emitted after the S-build so they don't block DVE's relu work) ----
        inp_pool = ctx.enter_context(tc.tile_pool(name="inp", bufs=1))
        for b in range(NB):
            stf = inp_pool.tile([P, NT, HID], F32, tag=f"stf{b}")
            xf = inp_pool.tile([P, NT, 2], F32, tag=f"xf{b}")
            nc.sync.dma_start(stf[:], st_d[b].rearrange("(t p) h -> p t h", p=P))
            nc.sync.dma_start(xf[:], x_d[b].rearrange("(t p) h -> p t h", p=P))
            nc.gpsimd.tensor_copy(
                x0_sb[:, :, b * CPB + 2 : b * CPB + 2 + HID], stf[:]
            )
            nc.gpsimd.tensor_copy(x0_sb[:, :, b * CPB : b * CPB + 2], xf[:])
            nc.gpsimd.tensor_copy(stt_sb[:, :, b, :], stf[:])

        # ---- S = exp(relu(E E^T)) with row sums; the first WAVE chain
        # groups of gconv1 consume S row-tiles as they land, filling the
        # PE while DVE-relu/ACT-exp pace the pipeline ----
        cpsum = ctx.enter_context(tc.tile_pool(name="cpsum", bufs=1, space="PSUM"))
        WAVE = 4
        wave_cp = {}
        for w in range(WAVE):
            wave_cp[w] = cpsum.tile(
                [P, NB * CH], F32, tag=f"wv{w}", name=f"wavecp{w}"
            )
        with (
            tc.tile_pool(name="lpsum", bufs=3, space="PSUM") as lpsum,
            tc.tile_pool(name="lrelu", bufs=3) as lrelu,
        ):
            for mt in range(NT):
                lr = lrelu.tile([P, N], F32)
                for q in range(4):
                    lp = lpsum.tile([P, 512], F32)
                    nc.tensor.matmul(
                        lp[:],
                        lhsT=etp[:, 0, mt * P : (mt + 1) * P],
                        rhs=etp[:, 1, q * 512 : (q + 1) * 512],
                        start=True,
                        stop=True,
                    )
                    nc.vector.tensor_scalar_max(
                        lr[:, q * 512 : (q + 1) * 512], lp[:], 0.0
                    )
                # exp + row-sum in one big ACT op
                nc.scalar.activation(
                    S_sb[:, mt, :], lr[:], AF.Exp,
                    accum_out=dtot[:, mt : mt + 1],
                )
                nc.vector.reciprocal(rinv[:, mt : mt + 1], dtot[:, mt : mt + 1])
                nc.vector.tensor_scalar_mul(
                    rinv2[:, mt : mt + 1], rinv[:, mt : mt + 1], 2.0
                )
                for w in range(WAVE):
                    nc.tensor.matmul(
                        wave_cp[w][:],
                        lhsT=S_sb[:, mt, w * P : (w + 1) * P],
                        rhs=_dv(x0_sb[:, mt, :])[:, :, 0:CH],
                        start=(mt == 0),
                        stop=(mt == NT - 1),
                    )

        for w in range(WAVE):
            nc.scalar.activation(
                _dv(u1_sb[:, w, :])[:, :, 0:CH], wave_cp[w][:],
                AF.Copy, scale=rinv[:, w : w + 1],
            )

        tpsum = ctx.enter_context(tc.tile_pool(name="tpsum", bufs=2, space="PSUM"))
        zpsum = ctx.enter_context(tc.tile_pool(name="zpsum", bufs=2, space="PSUM"))
        xgt_pool = ctx.enter_context(tc.tile_pool(name="xgt", bufs=3 * RT_GROUP))
        epi_pool = ctx.enter_context(tc.tile_pool(name="epi", bufs=6))

        def apply_S(src, dst, second, only_mt=None, mt0=0):
            """dst = (S @ src) / d   (or 2*(S @ src)/d - x0 when second)."""
            mts = range(mt0, NT) if only_mt is None else [only_mt]
            for mt in mts:
                cp = cpsum.tile([P, NB * CH], F32, tag=f"wv{mt % 4}", name=f"cp{mt}")
                for kt in range(NT):
                    nc.tensor.matmul(
                        cp[:],
                        lhsT=S_sb[:, kt, mt * P : (mt + 1) * P],
                        rhs=_dv(src[:, kt, :])[:, :, 0:CH],
                        start=(kt == 0),
                        stop=(kt == NT - 1),
                    )
                dstv = _dv(dst[:, mt, :])[:, :, 0:CH]
                if not second:
                    nc.scalar.activation(
                        dstv, cp[:], AF.Copy, scale=rinv[:, mt : mt + 1]
                    )
                else:
                    nc.vector.scalar_tensor_tensor(
                        out=dstv,
                        in0=cp[:],
                        scalar=rinv2[:, mt : mt + 1],
                        in1=_dv(x0_sb[:, mt, :])[:, :, 0:CH],
                        op0=ALU.mult,
                        op1=ALU.subtract,
                    )

        def tail_nt(nt, gate):
            """Transposes + W matmul + nonlinearity (+ epilogue) for one
            node tile (all 4 batches, batched PSUM + single wide ops)."""
            w_sb = wg_sb if gate else wu_sb
            oc = OC_G if gate else HID
            xgts = {}
            for b in range(NB):
                tp = tpsum.tile([TW, 3, P], BF16, tag="tp", name=f"tp{nt}{b}")
                for k, srcb in enumerate((x0_sb, u1_sb, u2_sb)):
                    nc.tensor.transpose(
                        tp[:, k, :],
                        srcb[:, nt, b * CPB : b * CPB + TW],
                        ident[:],
                    )
                xgt = xgt_pool.tile([TW, 3, P], BF16, tag="xgt", name=f"xg{nt}{b}")
                nc.vector.tensor_copy(xgt[:], tp[:])
                xgts[b] = xgt
            zp = zpsum.tile([P, NB, oc], F32, tag="zp", name=f"zp{nt}")
            for b in range(NB):
                for k in range(3):
                    nc.tensor.matmul(
                        zp[:, b, :],
                        lhsT=xgts[b][:, k, :],
                        rhs=w_sb[:, k, :],
                        start=(k == 0),
                        stop=(k == 2),
                    )
            if gate:
                nc.scalar.activation(zr_sb[:, nt], zp[:], AF.Sigmoid)
                # candidate: state-cols of x0 *= z  (in place, all b)
                x0c = _dv(x0_sb[:, nt, :])[:, :, 2 : 2 + HID]
                nc.vector.tensor_mul(x0c, x0c, zr_sb[:, nt, :, 0:HID])
            else:
                hc = epi_pool.tile([P, NB, HID], BF16, tag="hc", name=f"hc{nt}")
                nc.scalar.activation(hc[:], zp[:], AF.Tanh)
                r = zr_sb[:, nt, :, HID:OC_G]
                t1 = epi_pool.tile([P, NB, HID], BF16, tag="t1", name=f"t1{nt}")
                nc.vector.tensor_sub(t1[:], stt_sb[:, nt], hc[:])
                hf = epi_pool.tile([P, NB, HID], F32, tag="hf", name=f"hf{nt}")
                # h = hc + r*(state - hc)
                nc.vector.scalar_tensor_tensor(
                    out=hf[:], in0=t1[:], scalar=1.0, in1=r,
                    op0=ALU.mult, op1=ALU.mult,
                )
                nc.vector.tensor_add(hf[:], hf[:], hc[:])
                nc.sync.dma_start(
                    out_d[:, nt * P : (nt + 1) * P, :].rearrange(
                        "b p h -> p b h"
                    ),
                    hf[:],
                )

        # each gconv: first chain application, then the second application
        # interleaved per-mt with that node-tile's transpose/W-matmul tail
        # so the tail rides inside the chain's PE stream
        for gate in (True, False):
            apply_S(x0_sb, u1_sb, second=False, mt0=WAVE if gate else 0)
            for mt in range(NT):
                apply_S(u1_sb, u2_sb, second=True, only_mt=mt)
                tail_nt(mt, gate)


_NC = None


def _get_nc():
    global _NC
    if _NC is None:
        _NC = build_nc()
    return _NC


def _prep_in_maps(x, state, node_embeddings, W_gate, b_gate, W_update, b_update):
    bf = ml_dtypes.bfloat16
    x = np.asarray(x, dtype=np.float32)
    state = np.asarray(state, dtype=np.float32)
    E = np.asarray(node_embeddings, dtype=np.float32)
    W_gate = np.asarray(W_gate, dtype=np.float32)
    b_gate = np.asarray(b_gate, dtype=np.float32)
    W_update = np.asarray(W_update, dtype=np.float32)
    b_update = np.asarray(b_update, dtype=np.float32)

    eh = E.T.astype(bf)                       # [10, N] bf16
    el = (E.T - eh.astype(np.float32)).astype(bf)
    stack_l = np.concatenate([eh, el, eh], axis=0)   # lhsT rows
    stack_r = np.concatenate([eh, eh, el], axis=0)   # rhs rows
    et = np.ascontiguousarray(np.stack([stack_l, stack_r]))  # [2, 30, N]
    wg = np.zeros((3, 68, OC_G), np.float32)
    wu = np.zeros((3, 68, HID), np.float32)
    for k in range(3):
        wg[k, :CH] = W_gate[CH * k : CH * (k + 1)]
        wu[k, :CH] = W_update[CH * k : CH * (k + 1)]
    wg[2, CH] = b_gate
    wu[2, CH] = b_update
    wg = wg.astype(bf)
    wu = wu.astype(bf)

    ident = np.eye(P, dtype=bf)
    in_maps = []
    for r in range(NCORES):
        in_maps.append(
            {
                "x": np.ascontiguousarray(x[NB * r : NB * (r + 1)]),
                "state": np.ascontiguousarray(state[NB * r : NB * (r + 1)]),
                "et": et,
                "ident": ident,
                "wg": wg,
                "wu": wu,
            }
        )
    return in_maps


def run(trace=False, **inputs):
    nc = _get_nc()
    in_maps = _prep_in_maps(**inputs)
    res = run_bass_kernel_spmd(
        nc, in_maps, core_ids=list(range(NCORES)), trace=trace
    )
    out = np.concatenate([res.results[r]["out"] for r in range(NCORES)], axis=0)
    return out, res


def kernel(**inputs) -> np.ndarray:
    out, _ = run(trace=False, **inputs)
    return out
